# revision 50
# baseline (speedup 1.0000x reference)
"""LSH similarity-matrix kernel for Trainium2 (8 NeuronCores, data-parallel over batch).

Math: reference computes, per (l, b):
    c1 = (query_embed @ r.T > 0),  c2 = (doc_embed @ r.T > 0)   in {0,1}
    ham = s1 + s2 - 2*c1@c2.T ;  sim = cos(pi/NB * ham), masked where tok==0.
With +-1 codes U = 2c-1 and S = U1 @ U2.T:  ham = (NB - S)/2, so
    sim = sin(pi/(2*NB) * S).
Masks fold into the embeddings: a zeroed embedding row projects to 0,
sign(0) = 0 gives a zero code row, so S = 0 and sin(0) = 0 — exactly the
masked output. Masked doc tokens (half of them: tok in {0,1}) are gathered
away host-side entirely; output columns scatter back as zeros.

Sharding: batches are assigned to (core, slot) sorted by active-token
count; every slot is capped at 512 docs so each chunk's projection fits a
single PSUM bank. Docs beyond 512 (a few tens per heavy batch) ride in
fixed per-slot segments appended to the query columns of the shared
"aux" tile (query width + 2*overflow <= 512), so the SPMD program shape
is identical on every core and the overflow costs no extra projection,
sign instructions, or jobs — only a small extra dot+sin+store that runs
early, far off the critical tail.

Precision: the projection runs as a single float32r (TF32) matmul per
128-bit chunk. TF32's 11-bit mantissa flips ~1.4k of the 71M hash bits
(those whose fp32 projection sits within rounding error of zero), which
perturbs the final similarity by rel err ~7e-3 end-to-end — well inside
the 2e-2 gate — at 1/3 the PE cost of a compensated projection. The
embeddings and r stream from HBM straight into float32r tiles (f32r is
an engine compute mode over fp32 bits, so the DMA is a plain byte copy
and no on-device cast is needed). The code dot runs as fp8e4m3 DoubleRow
matmuls (chunk pairs give K=256 per MM at 0.5 cycles/row); +-1/0 codes
and their fp32 PSUM accumulation are exact.

The kernel is sign-throughput-bound: every projected bit crosses
PSUM->SBUF through DVE/ACT exactly once. So chunks are projected in
PAIRS into one 2-bank PSUM tile and signed by a single instruction
(halving per-instruction access overhead); pairs alternate between the
DVE (clamp) and ACT (Sign) engines, weighted so both engines carry equal
ns; aux (query+overflow) pairs interleave into the first jobs' slack. r is
pre-scaled by 2^66 host-side so the DVE clamp(x,-1,1) = max(min(x,1),-1)
sign is exact (any |proj| > 2^-66 maps to +-1). DMAs ride independent
queues (loads: SP/HWDGE, stores: Pool SWDGE, which also bypasses the
shared HWDGE dispatcher for the first doc load) so a store waiting on
Sin never blocks the next job's load.
"""
import os
import sys

sys.path.insert(0, "/opt/trn_rl_repo")

from contextlib import ExitStack

import numpy as np

import concourse.bass as bass
import concourse.mybir as mybir
import concourse.tile as tile
from concourse import bacc
from concourse.bass_utils import run_bass_kernel_spmd

L, BAT, A, BDOC, D, NB = 2, 32, 64, 1024, 128, 1024
CORES = 8
BPC = BAT // CORES          # batch slots per core
CH = NB // 128              # 8 bit-chunks
NPAIR = CH // 2             # chunk pairs
CAP = 512                   # per-slot doc cap (one PSUM bank)
SCALE = float(2.0 ** 66)
PI = float(np.pi)

F32 = mybir.dt.float32
F32R = mybir.dt.float32r
BF16 = mybir.dt.bfloat16
FP8 = mybir.dt.float8e4
Alu = mybir.AluOpType
Act = mybir.ActivationFunctionType
DR = mybir.MatmulPerfMode.DoubleRow

_BUILD_CACHE: dict = {}

# chunk-pair sign engine: 1 = DVE clamp, 0 = ACT Sign (ACT also runs Sin).
_PAIR_DVE = (1, 0, 1, 0)
_QPAIR_DVE = (0, 1, 1, 0)   # 2/2 split balances with the wider aux instrs


def _build(pads_c: tuple, qpad: int, seg_os: tuple, reps: int = 1):
    """Per-core SPMD program. pads_c[b] <= CAP: width of main slot b;
    seg_os[b]: overflow segment width of slot b (0 = none). reps > 1
    re-emits the whole body (timing instrumentation only)."""
    pads_c = tuple(int(p) for p in pads_c)
    seg_os = tuple(int(s) for s in seg_os)
    pad_cmax = max(pads_c)
    OV = sum(seg_os)
    seg_off = [sum(seg_os[:s]) for s in range(BPC)]
    OV2 = 2 * OV                    # both layers side by side
    assert OV2 <= 512, "overflow exceeds one PSUM bank"

    nc = bacc.Bacc("TRN2", target_bir_lowering=False, debug=False)

    QW = BPC * L * qpad
    W = QW + OV2                    # query columns + overflow doc columns
    assert W <= 512, "aux tile exceeds one PSUM bank"
    QE = nc.dram_tensor("qe", [D, W], F32R, kind="ExternalInput").ap()
    DE = nc.dram_tensor("de", [BPC, L, D, pad_cmax], F32R,
                        kind="ExternalInput").ap()
    RT = nc.dram_tensor("rt", [D, NB], F32R, kind="ExternalInput").ap()
    OUT = nc.dram_tensor("out", [BPC, L, qpad, pad_cmax], F32,
                         kind="ExternalOutput").ap()
    if OV:
        OUT2 = nc.dram_tensor("out2", [qpad, OV2], F32,
                              kind="ExternalOutput").ap()

    with tile.TileContext(nc) as tc, ExitStack() as ctx:
        const = ctx.enter_context(tc.tile_pool(name="const", bufs=1))
        jobp = ctx.enter_context(tc.tile_pool(name="jobp", bufs=4))
        outp = ctx.enter_context(tc.tile_pool(name="outp", bufs=4))
        # PSUM: chunk-pair tiles [128, 1024] (2 banks) x 3 bufs, plus the
        # dot-output tiles [*, 512] (1 bank) x 2 bufs = all 8 banks.
        ps_p = ctx.enter_context(tc.tile_pool(name="ps_p", bufs=3, space="PSUM"))

        for _rep in range(reps):
            _rp = f"r{_rep}_"
            # ---- constants, ordered for the serialized DMA-transfer queue:
            # SP/HWDGE carries the rt pieces (chunk 0-1 weights first so the
            # first projection unblocks earliest); the Pool SWDGE path
            # (bypasses the shared HWDGE dispatcher) carries the first doc
            # load + qe ----
            rt = const.tile([D, NB], F32R, tag="rt", name=f"{_rp}rt")
            nc.sync.dma_start(out=rt[:, 0:256], in_=RT[:, 0:256])
            nc.sync.dma_start(out=rt[:, 256:512], in_=RT[:, 256:512])
            qnat = const.tile([D, W], F32R, tag="qnat", name=f"{_rp}qnat")

            # PE pre-warm: dependency-free dummy matmuls run while the first
            # DMAs land their completion receipts, pulling the PE through its
            # cold/mid clock ramp so the real projections start at 2.4 GHz.
            # warm's memset rides the (idle until ~4.5us) DVE so the Pool
            # engine can start generating the first doc load immediately.
            warm = const.tile([D, 512], BF16, tag="warm", name=f"{_rp}warm")
            nc.vector.memset(warm, 0.0)
            wps = ps_p.tile([D, 1024], F32, tag="pp",
                            name=f"{_rp}wps")[:, 0:512]
            for i in range(4):
                nc.tensor.matmul(wps, warm[:, 0:128], warm,
                                 start=True, stop=True)

            def load_consts_tail():
                nc.sync.dma_start(out=qnat, in_=QE)
                nc.sync.dma_start(out=rt[:, 512:768], in_=RT[:, 512:768])
                nc.sync.dma_start(out=rt[:, 768:NB], in_=RT[:, 768:NB])

            U1 = const.tile([D, CH * W], FP8, tag="U1", name=f"{_rp}U1")

            def _pair_sign(pp, dst2, wcols, dve):
                """One instruction signs both chunks of a pair: pp cols
                [0:w] and [512:512+w] -> dst2 [p, 2, w]."""
                if wcols == 512:
                    sv = pp[:]                                  # [p, 1024]
                    dv = dst2
                else:
                    sv = pp[:].rearrange("p (h c) -> p h c",
                                         h=2)[:, :, 0:wcols]
                    dv = dst2.rearrange("p (h c) -> p h c", h=2)
                if dve:
                    nc.vector.tensor_scalar(dv, sv, 1.0, -1.0,
                                            Alu.min, Alu.max)
                else:
                    nc.scalar.activation(dv, sv, Act.Sign)

            def query_grp(g):
                # aux chunk pairs 2g, 2g+1 (query + overflow-doc columns in
                # one moving tile); emitted inside the first two jobs so the
                # sign work fills both engines' startup slack
                for pr in (2 * g, 2 * g + 1):
                    qp = ps_p.tile([D, 1024], F32, tag="pp",
                                   name=f"{_rp}qp{pr}")
                    for h in (0, 1):
                        k = 2 * pr + h
                        nc.tensor.matmul(qp[:, h * 512:h * 512 + W],
                                         rt[:, k * 128:(k + 1) * 128], qnat,
                                         start=True, stop=True)
                    _pair_sign(qp, U1[:, 2 * pr * W:(2 * pr + 2) * W],
                               W, _QPAIR_DVE[pr])

            # ---- doc jobs, software-pipelined emission ----
            jobs = [(b, l) for b in range(BPC) for l in range(L)]
            n = len(jobs)
            st = [dict() for _ in range(n + 1)]     # [-1] = overflow job

            def stage_a(j):
                b, l = jobs[j]
                pad_c = pads_c[b]
                dnat = jobp.tile([D, pad_cmax], F32R, tag="dnat",
                                 name=f"{_rp}dnat{j}")[:, 0:pad_c]
                # job 0's load takes the Pool SWDGE path: it runs
                # concurrently with the rt pieces on SP
                eng = nc.gpsimd if j == 0 else nc.sync
                eng.dma_start(out=dnat, in_=DE[b, l, :, 0:pad_c])
                st[j]["e"] = dnat

            def stage_b(j, prs=None):
                b, l = jobs[j]
                pad_c = pads_c[b]
                ev = st[j]["e"]
                if prs is None or prs[0] == 0:
                    st[j]["U2"] = jobp.tile([D, CH * pad_cmax], FP8, tag="U2",
                                            name=f"{_rp}U2{j}")
                U2 = st[j]["U2"]
                for pr in (range(NPAIR) if prs is None else prs):
                    pp = ps_p.tile([D, 1024], F32, tag="pp",
                                   name=f"{_rp}pp{j}_{pr}")
                    for h in (0, 1):
                        k = 2 * pr + h
                        nc.tensor.matmul(pp[:, h * 512:h * 512 + pad_c],
                                         rt[:, k * 128:(k + 1) * 128], ev,
                                         start=True, stop=True)
                    _pair_sign(pp, U2[:, 2 * pr * pad_c:(2 * pr + 2) * pad_c],
                               pad_c, _PAIR_DVE[pr])

            def _dot(S, U2, pad_c, qcol, c0, c1, p0):
                for jj in range(NPAIR):
                    lw = U1[:, 2 * jj * W:(2 * jj + 2) * W] \
                        .rearrange("p (o c) -> p o c", o=2)[:, :, qcol:qcol + qpad]
                    rv = U2[:, 2 * jj * pad_c:(2 * jj + 2) * pad_c] \
                        .rearrange("p (o c) -> p o c", o=2)[:, :, c0:c1]
                    nc.tensor.matmul(S[:, p0:p0 + c1 - c0], lw, rv,
                                     start=(jj == 0), stop=(jj == NPAIR - 1),
                                     perf_mode=DR)

            def stage_c(j, tail=False):
                b, l = jobs[j]
                pad_c = pads_c[b]
                U2 = st[j]["U2"]
                qcol = (b * L + l) * qpad
                sim = outp.tile([qpad, pad_cmax], F32, tag="sim",
                                name=f"{_rp}sim{j}")[:, 0:pad_c]
                if tail:
                    # last job (no overflow): per-half dot->sin->store with
                    # separate S tiles so the final store isn't serialized
                    # behind the full-width sin
                    hw2 = pad_c // 2
                    for pi, (c0, c1) in enumerate([(0, hw2), (hw2, pad_c)]):
                        S = ps_p.tile([qpad, 512], F32, tag="s",
                                      bufs=2, name=f"{_rp}St{pi}")
                        _dot(S, U2, pad_c, qcol, c0, c1, 0)
                        nc.scalar.activation(sim[:, c0:c1], S[:, 0:c1 - c0],
                                             Act.Sin, scale=PI / (2.0 * NB))
                        eng = nc.sync if pi else nc.gpsimd
                        eng.dma_start(out=OUT[b, l, :, c0:c1],
                                      in_=sim[:, c0:c1])
                    return
                S = ps_p.tile([qpad, 512], F32, tag="s", bufs=2,
                              name=f"{_rp}S{j}")
                _dot(S, U2, pad_c, qcol, 0, pad_c, 0)
                nc.scalar.activation(sim, S[:, 0:pad_c], Act.Sin,
                                     scale=PI / (2.0 * NB))
                # stores ride the Pool SWDGE queue (never block loads); the
                # final job's store takes SP's lower-latency hwdge path
                eng = nc.sync if j == n - 1 else nc.gpsimd
                eng.dma_start(out=OUT[b, l, :, 0:pad_c], in_=sim)

            # ---- overflow docs live in the aux tile's columns [QW:W]:
            # their codes are produced by the aux pair signs, so only the
            # dot+sin+store remain — emitted early, right after c(0), far
            # off the tail ----
            def stage_c_ov():
                S = ps_p.tile([qpad, 512], F32, tag="s", bufs=2,
                              name=f"{_rp}Sov")
                sim = outp.tile([qpad, pad_cmax], F32, tag="sim",
                                name=f"{_rp}simov")[:, 0:OV2]
                for li in range(L):
                    for s in range(BPC):
                        if not seg_os[s]:
                            continue
                        c0 = li * OV + seg_off[s]
                        qcol = (s * L + li) * qpad
                        _dot(S, U1, W, qcol, QW + c0, QW + c0 + seg_os[s], c0)
                nc.scalar.activation(sim, S[:, 0:OV2], Act.Sin,
                                     scale=PI / (2.0 * NB))
                nc.gpsimd.dma_start(out=OUT2, in_=sim)

            stage_a(0)
            load_consts_tail()
            # job 0's pairs interleave with the query pairs so both sign
            # engines run continuously from the first DMA landing: pairs
            # 0-1 + query chunks 0-3 need only rt[:512]+qe; the rest rt[512:]
            stage_b(0, prs=(0, 1))
            query_grp(0)
            stage_b(0, prs=(2, 3))
            query_grp(1)
            stage_a(1)
            stage_a(2)
            stage_b(1)
            stage_a(3)
            for j in range(n):
                if j + 2 < n:
                    stage_b(j + 2)
                stage_c(j, tail=False)
                if OV and j == 0:
                    stage_c_ov()
                if j + 4 < n:
                    stage_a(j + 4)

    nc.compile()
    return nc


def _stage_inputs(query_embed, doc_embed, query_tok, doc_tok, r):
    query_embed = np.ascontiguousarray(query_embed, dtype=np.float32)
    doc_embed = np.ascontiguousarray(doc_embed, dtype=np.float32)
    r = np.ascontiguousarray(r, dtype=np.float32)

    qmask = (np.asarray(query_tok) != 0)
    dmask = (np.asarray(doc_tok) != 0)

    # sort batches by active count; slot s takes ranks [s*CORES, (s+1)*CORES)
    # spread across the 8 cores, so per-slot padding is tight and identical
    # on every core (SPMD requires one shape per slot)
    counts = dmask.sum(axis=1).astype(int)
    order = np.argsort(counts, kind="stable")
    assign = np.empty((CORES, BPC), dtype=int)   # assign[c, b] = batch id
    for s in range(BPC):
        for c in range(CORES):
            assign[c, s] = order[s * CORES + c]
    maxes = [int(counts[assign[:, s]].max()) for s in range(BPC)]
    pads_c = tuple(min(CAP, max(64, -(-m // 32) * 32)) for m in maxes)
    seg_os = tuple(-(-max(0, m - CAP) // 8) * 8 for m in maxes)
    pad_cmax = max(pads_c)
    OV = sum(seg_os)
    seg_off = [sum(seg_os[:s]) for s in range(BPC)]

    qe_m = query_embed * qmask[None, :, :, None].astype(np.float32)
    qidxs = [np.flatnonzero(qmask[g]) for g in range(BAT)]
    qpad = min(A, max(16, int(-(-max(len(q) for q in qidxs) // 8) * 8)))
    QW = BPC * L * qpad

    rts = np.ascontiguousarray(r.T * SCALE)          # [D, NB], fp32 bits

    idxs = [np.flatnonzero(dmask[g]) for g in range(BAT)]
    in_maps = []
    for c in range(CORES):
        # embeddings staged pre-transposed [D, tokens]; queries compacted
        # to their active rows (masks are per-batch, shared by both layers)
        # aux = compacted queries followed by the overflow-doc segments
        qe_c = np.zeros((D, QW + 2 * OV), dtype=np.float32)
        de_c = np.zeros((BPC, L, D, pad_cmax), dtype=np.float32)
        for b in range(BPC):
            g = assign[c, b]
            qi = qidxs[g]
            for li in range(L):
                col = (b * L + li) * qpad
                qe_c[:, col:col + len(qi)] = qe_m[li, g, qi].T
            idx = idxs[g][:CAP]
            de_c[b, :, :, :len(idx)] = doc_embed[:, g, idx].transpose(0, 2, 1)
            ovi = idxs[g][CAP:]
            if len(ovi):
                for li in range(L):
                    c0 = QW + li * OV + seg_off[b]
                    qe_c[:, c0:c0 + len(ovi)] = doc_embed[li, g, ovi].T
        in_maps.append({"qe": qe_c, "de": de_c, "rt": rts})

    return in_maps, assign, idxs, pads_c, seg_os, qidxs, qpad


def kernel(query_embed, doc_embed, query_tok, doc_tok, r):
    in_maps, assign, idxs, pads_c, seg_os, qidxs, qpad = _stage_inputs(
        query_embed, doc_embed, query_tok, doc_tok, r)
    OV = sum(seg_os)
    seg_off = [sum(seg_os[:s]) for s in range(BPC)]

    key = (pads_c, qpad, seg_os)
    if key not in _BUILD_CACHE:
        _BUILD_CACHE[key] = _build(pads_c, qpad, seg_os)
    nc = _BUILD_CACHE[key]

    res = run_bass_kernel_spmd(nc, in_maps, core_ids=list(range(CORES)))

    out = np.zeros((BAT, L, A, BDOC), dtype=np.float32)
    for c in range(CORES):
        o_c = res.results[c]["out"]  # [BPC, L, qpad, pad_cmax]
        o2_c = res.results[c].get("out2")
        for b in range(BPC):
            g = assign[c, b]
            idx = idxs[g][:CAP]
            qi = qidxs[g]
            for li in range(L):
                out[g, li][np.ix_(qi, idx)] = o_c[b, li, :len(qi), :len(idx)]
            ovi = idxs[g][CAP:]
            if len(ovi):
                for li in range(L):
                    c0 = li * OV + seg_off[b]
                    out[g, li][np.ix_(qi, ovi)] = \
                        o2_c[:len(qi), c0:c0 + len(ovi)]
    return out
